# revision 3
# baseline (speedup 1.0000x reference)
"""MinGRU (B=4, T=4096, D=1024) Trainium2 kernel, 8-core SPMD.

Sharding: core i handles (batch b = i//2, output-channel half j = i%2).
Each core computes u_z = x[b] @ Wz[half].T, u_h = x[b] @ Wh[half].T,
z = sigmoid(u_z + bz), a = 1 - z, bvec = z * (u_h + bh), then the
recurrence h_t = a_t * h_{t-1} + b_t via the hardware tensor_tensor_scan.

Matmuls run in fp8 (e4m3) DoubleRow mode: each DR instruction carries two
independent (weights, moving) slot pairs and processes a 512-wide moving
tile in 256 PE cycles, i.e. 4x the bf16 column rate. e4m3's 3-bit mantissa
alone is out of error budget, so each product is computed as three slot
series accumulating into the same PSUM region:
  A: e4m3(S*W) . xh          (main term, S = 512)
  B: e4m3(S*W - A) . xh      (weight-quantization correction)
  C: e4m3(W) . xl            (x-quantization correction, xl = e4m3(S*(x-xh)))
Series pair adjacent k-tiles into the two DR slots. Total PE work is 6
series x 8 k-tiles = 24 DR instructions per 512 cols vs 16 bf16 matmuls:
0.75x the baseline bf16 cycles, with end-to-end rel err ~2e-3 (vs 3.3e-3
for bf16). PSUM holds S*u; ACT folds the 1/S into the sigmoid scale and
the scan output S*h is divided by S on the host (the scan is linear in b).

a = 1 - z is computed on ACT as sigmoid(-u - bz), moving a [128,1024] op
from DVE (the next-busiest engine) to ACT. DVE does bv = (ph + S*bh) * z
and the scan (fp32 state, bf16 out).

Other structure (x fully resident in SBUF, one PSUM supertile of 2 banks
per tensor, _dedupe_ldweights across the two 512-col halves,
For_i(staggered_reset=True)) follows the bf16 baseline, which A/B-tested
these choices on hardware.
"""

import numpy as np

_B, _T, _D = 4, 4096, 1024
_EH = 512
_NG = _EH // 128
_TT = 1024         # timestep supertile (2 PSUM banks)
_NT = _T // _TT    # 4 resident t-supertiles
_NK = _D // 128
_HF = 512
_S = 512.0         # global fp8 scale: PSUM = S * u
_NTERM = 3         # slot series per matmul (A, B, C above)
_ZSER = ("A", "B", "C")   # z-path series
_HSER = ("A", "B", "C")   # h-path series


def _dedupe_ldweights(nc):
    from concourse import mybir as mb

    removed = 0
    for fn in nc.m.functions:
        for blk in fn.blocks:
            insts = blk.instructions
            last_sig = None
            dels = []
            for i, inst in enumerate(insts):
                if isinstance(inst, mb.InstLdweights):
                    ap = inst.ins[0]
                    sig = (str(ap.memref), ap.offset, str(ap.ap),
                           str(ap.dtype), inst.perf_mode, inst.tile_position)
                    si = inst.sync_info
                    clean = si is None or (not si.on_wait and not si.on_update)
                    if sig == last_sig and clean:
                        dels.append(i)
                    else:
                        last_sig = sig
                elif isinstance(inst, mb.InstMatmult):
                    if inst.ldweights is not False:
                        last_sig = None
            for i in reversed(dels):
                del insts[i]
            removed += len(dels)
    return removed


def _build(reps=1, loop_n=None):
    from contextlib import ExitStack
    from concourse import bacc, mybir, tile

    f32 = mybir.dt.float32
    bf16 = mybir.dt.bfloat16
    fp8 = mybir.dt.float8e4
    AF = mybir.ActivationFunctionType
    OP = mybir.AluOpType
    DR = mybir.MatmulPerfMode.DoubleRow

    nc = bacc.Bacc("TRN2", debug=False, num_devices=8)
    # x: row d = k*128+p, cols (s, t): s=0 -> xh = e4m3(x), s=1 -> xl.
    xt = nc.dram_tensor("xt", [_D, 2 * _T], fp8, kind="ExternalInput").ap()
    # weights: row d, cols (g, term, m): term 0/1/2 = A/B/C series.
    wzt = nc.dram_tensor("wzt", [_D, _NG * _NTERM * 128], fp8,
                         kind="ExternalInput").ap()
    wht = nc.dram_tensor("wht", [_D, _NG * _NTERM * 128], fp8,
                         kind="ExternalInput").ap()
    bzt = nc.dram_tensor("bzt", [128, _NG], f32, kind="ExternalInput").ap()
    bht = nc.dram_tensor("bht", [128, _NG], f32, kind="ExternalInput").ap()
    hout = nc.dram_tensor("h", [_EH, _T], bf16, kind="ExternalOutput").ap()

    _WF = _NG * _NTERM * 128  # weight free size per k-tile

    with tile.TileContext(nc) as tc, ExitStack() as ctx:
        wpool = ctx.enter_context(tc.tile_pool(name="w", bufs=1))
        vpool = ctx.enter_context(tc.tile_pool(name="v", bufs=3))
        hpool = ctx.enter_context(tc.tile_pool(name="h", bufs=2))
        ppool = ctx.enter_context(tc.tile_pool(name="p", bufs=2, space="PSUM"))

        # x fully resident: 4 supertiles of [128, (k s t)] fp8 = 16 KB/part
        # each (64 KB/part total).
        xres = []
        for t2 in range(_NT):
            xres_t = wpool.tile([128, _NK * 2 * _TT], fp8, tag=f"x{t2}")
            xres.append(xres_t)
        wz_sb = wpool.tile([128, _NK * _WF], fp8, tag="wz")
        wh_sb = wpool.tile([128, _NK * _WF], fp8, tag="wh")
        bz_sb = wpool.tile([128, _NG], f32, tag="bz")
        bzn_sb = wpool.tile([128, _NG], f32, tag="bzn")
        bh_sb = wpool.tile([128, _NG], f32, tag="bh")

        def x_chunk(t2, ks, nk):
            for s in range(2):
                nc.sync.dma_start(
                    xres[t2][:, ks * 2 * _TT:(ks + nk) * 2 * _TT].rearrange(
                        "p (k s t) -> p k s t", k=nk, s=2)[:, :, s],
                    xt.rearrange("(k p) (s t) -> p k s t", p=128, s=2)[
                        :, ks:ks + nk, s, t2 * _TT:(t2 + 1) * _TT],
                )

        def w_chunk(k):
            nc.sync.dma_start(
                wz_sb[:, k * _WF:(k + 1) * _WF],
                wzt[k * 128:(k + 1) * 128, :])
            nc.sync.dma_start(
                wh_sb[:, k * _WF:(k + 1) * _WF],
                wht[k * 128:(k + 1) * 128, :])

        x_chunk(0, 0, 4)
        w_chunk(0)
        w_chunk(1)
        nc.sync.dma_start(bz_sb[:], bzt)
        nc.sync.dma_start(bh_sb[:], bht)
        x_chunk(0, 4, 4)
        for k in range(2, _NK):
            w_chunk(k)
        for t2 in range(1, _NT):
            x_chunk(t2, 0, 4)
            x_chunk(t2, 4, 4)
        nc.scalar.mul(bzn_sb[:], bz_sb[:], -1.0)

        # 5D views: weights [p, k, g, term, m], x [p, k, s, t]
        wz5 = wz_sb[:].rearrange("p (k g u m) -> p k g u m",
                                 k=_NK, g=_NG, u=_NTERM)
        wh5 = wh_sb[:].rearrange("p (k g u m) -> p k g u m",
                                 k=_NK, g=_NG, u=_NTERM)

        def emit_matmuls(psum, w5, x4, g, series):
            """One u = x @ W.T product: DR slot series into psum[:, c0:]."""
            n = len(series) * (_NK // 2)
            i = 0
            for s in series:
                term, xs = {"A": (0, 0), "B": (1, 0), "C": (2, 1)}[s]
                for kp in range(_NK // 2):
                    for c0 in (0, _HF):
                        nc.tensor.matmul(
                            psum[:, c0:c0 + _HF],
                            lhsT=w5[:, 2 * kp:2 * kp + 2, g, term, :],
                            rhs=x4[:, 2 * kp:2 * kp + 2, xs, c0:c0 + _HF],
                            start=(i == 0),
                            stop=(i == n - 1),
                            perf_mode=DR,
                        )
                    i += 1

        def body(first):
          hprev = [None] * _NG
          for t2 in range(_NT):
            x4 = xres[t2][:].rearrange("p (k s t) -> p k s t", k=_NK, s=2)
            for g in range(_NG):
                last = (t2 == _NT - 1 and g == _NG - 1)
                pz = ppool.tile([128, _TT], f32, tag="pz")
                ph = ppool.tile([128, _TT], f32, tag="ph")
                emit_matmuls(pz, wz5, x4, g, _ZSER)
                emit_matmuls(ph, wh5, x4, g, _HSER)
                z = vpool.tile([128, _TT], f32, tag="z")
                av = vpool.tile([128, _TT], f32, tag="a")
                bv = vpool.tile([128, _TT], f32, tag="b")
                hb = hpool.tile([128, _TT], bf16, tag=f"h{g}")
                init = 0.0 if hprev[g] is None \
                    else hprev[g][:, _TT - 1:_TT]
                halves = ((0, _HF), (_HF, _HF)) if last else ((0, _TT),)
                for (c0, w) in halves:
                    sl = slice(c0, c0 + w)
                    nc.scalar.activation(z[:, sl], pz[:, sl], AF.Sigmoid,
                                         bias=bz_sb[:, g:g + 1],
                                         scale=1.0 / _S)
                    nc.scalar.activation(av[:, sl], pz[:, sl], AF.Sigmoid,
                                         bias=bzn_sb[:, g:g + 1],
                                         scale=-1.0 / _S)
                    nc.vector.scalar_tensor_tensor(
                        bv[:, sl], ph[:, sl], bh_sb[:, g:g + 1], z[:, sl],
                        OP.add, OP.mult)
                    nc.vector.tensor_tensor_scan(
                        hb[:, sl], av[:, sl], bv[:, sl], init,
                        OP.mult, OP.add)
                    init = hb[:, c0 + w - 1:c0 + w]
                    nc.sync.dma_start(
                        hout[g * 128:(g + 1) * 128,
                             t2 * _TT + c0: t2 * _TT + c0 + w],
                        hb[:, sl])
                hprev[g] = hb

        if loop_n is not None:
            body(True)
            from concourse import mybir as _mb
            with tc.For_i(0, loop_n, 1, hint_engines=(
                    _mb.EngineType.PE, _mb.EngineType.SP,
                    _mb.EngineType.DVE, _mb.EngineType.Activation),
                    staggered_reset=True):
                body(False)
        else:
            for rep in range(reps):
                body(rep == 0)

    _dedupe_ldweights(nc)
    nc.compile()
    return nc


_NC_CACHE = None


def _shard_inputs(inputs):
    import ml_dtypes

    e4 = ml_dtypes.float8_e4m3
    x = np.asarray(inputs["x"], dtype=np.float32)
    Wz = np.asarray(inputs["Wz"], dtype=np.float32)
    bz = np.asarray(inputs["bz"], dtype=np.float32)
    Wh = np.asarray(inputs["Wh"], dtype=np.float32)
    bh = np.asarray(inputs["bh"], dtype=np.float32)

    def w_pack(W, j):
        # W is (out, in) torch-style; take output half j, transpose to
        # (in 1024, out 512), build [1024, g, term, m] slot series.
        WT = np.ascontiguousarray(W[j * _EH:(j + 1) * _EH, :].T)
        wA = (WT * _S).astype(e4)
        wB = (WT * _S - wA.astype(np.float32)).astype(e4)
        w1 = WT.astype(e4)
        arr = np.empty((_D, _NG, _NTERM, 128), dtype=e4)
        arr[:, :, 0, :] = wA.reshape(_D, _NG, 128)
        arr[:, :, 1, :] = wB.reshape(_D, _NG, 128)
        arr[:, :, 2, :] = w1.reshape(_D, _NG, 128)
        return np.ascontiguousarray(arr.reshape(_D, _NG * _NTERM * 128))

    in_maps = []
    xt_by_batch = {}
    for b in range(_B):
        xT = np.ascontiguousarray(x[b].T)        # (D, T)
        xh = xT.astype(e4)
        xl = ((xT - xh.astype(np.float32)) * _S).astype(e4)
        xt = np.empty((_D, 2, _T), dtype=e4)
        xt[:, 0, :] = xh
        xt[:, 1, :] = xl
        xt_by_batch[b] = np.ascontiguousarray(xt.reshape(_D, 2 * _T))
    wz_by_half = {j: w_pack(Wz, j) for j in range(2)}
    wh_by_half = {j: w_pack(Wh, j) for j in range(2)}

    for i in range(8):
        b, j = i // 2, i % 2
        sl = slice(j * _EH, (j + 1) * _EH)
        in_maps.append({
            "xt": xt_by_batch[b],
            "wzt": wz_by_half[j],
            "wht": wh_by_half[j],
            "bzt": np.ascontiguousarray(bz[sl].reshape(_NG, 128).T),
            "bht": np.ascontiguousarray(
                (bh[sl] * _S).reshape(_NG, 128).T),
        })
    return in_maps


def run(inputs, trace=False, tmpdir=None):
    global _NC_CACHE
    from concourse.bass_utils import run_bass_kernel_spmd

    if _NC_CACHE is None:
        _NC_CACHE = _build()
    nc = _NC_CACHE
    in_maps = _shard_inputs(inputs)
    res = run_bass_kernel_spmd(
        nc, in_maps, core_ids=list(range(8)), trace=trace, tmpdir=tmpdir)
    out = np.empty((_B, _T, _D), dtype=np.float32)
    inv_s = np.float32(1.0 / _S)
    for i in range(8):
        b, j = i // 2, i % 2
        out[b, :, j * _EH:(j + 1) * _EH] = \
            res.results[i]["h"].astype(np.float32).T * inv_s
    return out, res


def kernel(**inputs):
    out, _ = run(inputs, trace=False)
    return out


# revision 5
# speedup vs baseline: 1.4277x; 1.4277x over previous
"""MinGRU (B=4, T=4096, D=1024) Trainium2 kernel, 8-core SPMD.

Sharding: core i handles (batch b = i//2, output-channel half j = i%2).
Each core computes u_z = x[b] @ Wz[half].T, u_h = x[b] @ Wh[half].T,
z = sigmoid(u_z + bz), a = 1 - z, bvec = z * (u_h + bh), then the
recurrence h_t = a_t * h_{t-1} + b_t via the hardware tensor_tensor_scan.

Design (all empirically A/B-tested on hardware):
- bf16 matmuls: reads half the SBUF bytes per MAC of fp32r, which holds
  speed under the chip's power-driven clock governor where fp32r drifts
  up (interleaved A/B: bf16 149-152us vs fp32r 159-166us).
- Each 128x128 weight tile streams against two 512-col moving tiles
  into a [128,1024] 2-bank PSUM supertile; _dedupe_ldweights deletes
  the redundant second InstLdweights post-legalization (the in-order PE
  reuses the resident stationary operand - verified bit-correct).
- x fully resident in SBUF (64 KB/partition), loaded once at startup:
  the steady-state loop's only DMA is the bf16 h output.
- One PSUM reader per bank: ACT does z = sigmoid(pz + bz); DVE does
  a = 1 - z, b = (ph + bh) * z, and the scan (fp32 state, bf16 out).
- For_i(staggered_reset=True) removes the per-iteration all-engine
  barrier.
Host pre-transposes x and W so every DMA is row-contiguous and converts
h back to fp32.
"""

import numpy as np

_B, _T, _D = 4, 4096, 1024
_EH = 512
_NG = _EH // 128
_TT = 1024         # timestep supertile (2 PSUM banks)
_NT = _T // _TT    # 4 resident t-supertiles
_NK = _D // 128
_HF = 512


def _dedupe_ldweights(nc):
    from concourse import mybir as mb

    removed = 0
    for fn in nc.m.functions:
        for blk in fn.blocks:
            insts = blk.instructions
            last_sig = None
            dels = []
            for i, inst in enumerate(insts):
                if isinstance(inst, mb.InstLdweights):
                    ap = inst.ins[0]
                    sig = (str(ap.memref), ap.offset, str(ap.ap),
                           str(ap.dtype), inst.perf_mode, inst.tile_position)
                    si = inst.sync_info
                    clean = si is None or (not si.on_wait and not si.on_update)
                    if sig == last_sig and clean:
                        dels.append(i)
                    else:
                        last_sig = sig
                elif isinstance(inst, mb.InstMatmult):
                    if inst.ldweights is not False:
                        last_sig = None
            for i in reversed(dels):
                del insts[i]
            removed += len(dels)
    return removed


def _build(reps=1, loop_n=None):
    from contextlib import ExitStack
    from concourse import bacc, mybir, tile

    f32 = mybir.dt.float32
    bf16 = mybir.dt.float16
    AF = mybir.ActivationFunctionType
    OP = mybir.AluOpType

    nc = bacc.Bacc("TRN2", debug=False, num_devices=8)
    xt = nc.dram_tensor("xt", [_D, _T], bf16, kind="ExternalInput").ap()
    wzt = nc.dram_tensor("wzt", [_D, _EH], bf16, kind="ExternalInput").ap()
    wht = nc.dram_tensor("wht", [_D, _EH], bf16, kind="ExternalInput").ap()
    bzt = nc.dram_tensor("bzt", [128, _NG], f32, kind="ExternalInput").ap()
    bht = nc.dram_tensor("bht", [128, _NG], f32, kind="ExternalInput").ap()
    hout = nc.dram_tensor("h", [_EH, _T], bf16, kind="ExternalOutput").ap()

    with tile.TileContext(nc) as tc, ExitStack() as ctx:
        wpool = ctx.enter_context(tc.tile_pool(name="w", bufs=1))
        vpool = ctx.enter_context(tc.tile_pool(name="v", bufs=3))
        hpool = ctx.enter_context(tc.tile_pool(name="h", bufs=2))
        ppool = ctx.enter_context(tc.tile_pool(name="p", bufs=2, space="PSUM"))

        # x fully resident: 4 supertiles of [128, (k tt)] bf16 = 64 KB/part.
        xres = []
        for t2 in range(_NT):
            xres_t = wpool.tile([128, _NK * _TT], bf16, tag=f"x{t2}")
            xres.append(xres_t)
        wz_sb = wpool.tile([128, _NK * _EH], bf16, tag="wz")
        wh_sb = wpool.tile([128, _NK * _EH], bf16, tag="wh")
        bz_sb = wpool.tile([128, _NG], f32, tag="bz")
        bh_sb = wpool.tile([128, _NG], f32, tag="bh")

        def x_chunk(t2, ks, nk):
            nc.sync.dma_start(
                xres[t2][:, ks * _TT:(ks + nk) * _TT].rearrange(
                    "p (k t) -> p k t", k=nk),
                xt.rearrange("(k p) t -> p k t", p=128)[
                    :, ks:ks + nk, t2 * _TT:(t2 + 1) * _TT],
            )

        def w_chunk(k):
            nc.sync.dma_start(
                wz_sb[:, k * _EH:(k + 1) * _EH],
                wzt[k * 128:(k + 1) * 128, :])
            nc.sync.dma_start(
                wh_sb[:, k * _EH:(k + 1) * _EH],
                wht[k * 128:(k + 1) * 128, :])

        x_chunk(0, 0, 4)
        w_chunk(0)
        w_chunk(1)
        nc.sync.dma_start(bz_sb[:], bzt)
        nc.sync.dma_start(bh_sb[:], bht)
        x_chunk(0, 4, 4)
        for k in range(2, _NK):
            w_chunk(k)
        for t2 in range(1, _NT):
            x_chunk(t2, 0, 4)
            x_chunk(t2, 4, 4)

        def body(first):
          hprev = [None] * _NG
          for t2 in range(_NT):
            xs = xres[t2]
            for g in range(_NG):
                last = (t2 == _NT - 1 and g == _NG - 1)
                pz = ppool.tile([128, _TT], f32, tag="pz")
                ph = ppool.tile([128, _TT], f32, tag="ph")
                for k in range(_NK):
                    for c0 in (0, _HF):
                        nc.tensor.matmul(
                            pz[:, c0:c0 + _HF],
                            lhsT=wz_sb[:, k * _EH + g * 128:
                                       k * _EH + (g + 1) * 128],
                            rhs=xs[:, k * _TT + c0: k * _TT + c0 + _HF],
                            start=(k == 0),
                            stop=(k == _NK - 1),
                        )
                for k in range(_NK):
                    for c0 in (0, _HF):
                        nc.tensor.matmul(
                            ph[:, c0:c0 + _HF],
                            lhsT=wh_sb[:, k * _EH + g * 128:
                                       k * _EH + (g + 1) * 128],
                            rhs=xs[:, k * _TT + c0: k * _TT + c0 + _HF],
                            start=(k == 0),
                            stop=(k == _NK - 1),
                        )
                z = vpool.tile([128, _TT], f32, tag="z")
                av = vpool.tile([128, _TT], f32, tag="a")
                bv = vpool.tile([128, _TT], f32, tag="b")
                hb = hpool.tile([128, _TT], bf16, tag=f"h{g}")
                init = 0.0 if hprev[g] is None \
                    else hprev[g][:, _TT - 1:_TT]
                halves = ((0, _HF), (_HF, _HF)) if last else ((0, _TT),)
                for (c0, w) in halves:
                    sl = slice(c0, c0 + w)
                    nc.scalar.activation(z[:, sl], pz[:, sl], AF.Sigmoid,
                                         bias=bz_sb[:, g:g + 1])
                    nc.vector.tensor_scalar(
                        av[:, sl], z[:, sl], -1.0, 1.0, OP.mult, OP.add)
                    nc.vector.scalar_tensor_tensor(
                        bv[:, sl], ph[:, sl], bh_sb[:, g:g + 1], z[:, sl],
                        OP.add, OP.mult)
                    nc.vector.tensor_tensor_scan(
                        hb[:, sl], av[:, sl], bv[:, sl], init,
                        OP.mult, OP.add)
                    init = hb[:, c0 + w - 1:c0 + w]
                    nc.sync.dma_start(
                        hout[g * 128:(g + 1) * 128,
                             t2 * _TT + c0: t2 * _TT + c0 + w],
                        hb[:, sl])
                hprev[g] = hb

        if loop_n is not None:
            body(True)
            from concourse import mybir as _mb
            with tc.For_i(0, loop_n, 1, hint_engines=(
                    _mb.EngineType.PE, _mb.EngineType.SP,
                    _mb.EngineType.DVE, _mb.EngineType.Activation),
                    staggered_reset=True):
                body(False)
        else:
            for rep in range(reps):
                body(rep == 0)

    _dedupe_ldweights(nc)
    nc.compile()
    return nc


_NC_CACHE = None


def _shard_inputs(inputs):
    bf16 = np.float16
    x = np.asarray(inputs["x"], dtype=np.float32)
    Wz = np.asarray(inputs["Wz"], dtype=np.float32)
    bz = np.asarray(inputs["bz"], dtype=np.float32)
    Wh = np.asarray(inputs["Wh"], dtype=np.float32)
    bh = np.asarray(inputs["bh"], dtype=np.float32)

    wzT = np.ascontiguousarray(Wz.T).astype(bf16)
    whT = np.ascontiguousarray(Wh.T).astype(bf16)

    in_maps = []
    for i in range(8):
        b, j = i // 2, i % 2
        sl = slice(j * _EH, (j + 1) * _EH)
        in_maps.append({
            "xt": np.ascontiguousarray(x[b].T).astype(bf16),
            "wzt": np.ascontiguousarray(wzT[:, sl]),
            "wht": np.ascontiguousarray(whT[:, sl]),
            "bzt": np.ascontiguousarray(bz[sl].reshape(_NG, 128).T),
            "bht": np.ascontiguousarray(bh[sl].reshape(_NG, 128).T),
        })
    return in_maps


def run(inputs, trace=False, tmpdir=None):
    global _NC_CACHE
    from concourse.bass_utils import run_bass_kernel_spmd

    if _NC_CACHE is None:
        _NC_CACHE = _build()
    nc = _NC_CACHE
    in_maps = _shard_inputs(inputs)
    res = run_bass_kernel_spmd(
        nc, in_maps, core_ids=list(range(8)), trace=trace, tmpdir=tmpdir)
    out = np.empty((_B, _T, _D), dtype=np.float32)
    for i in range(8):
        b, j = i // 2, i % 2
        out[b, :, j * _EH:(j + 1) * _EH] = res.results[i]["h"].astype(np.float32).T
    return out, res


def kernel(**inputs):
    out, _ = run(inputs, trace=False)
    return out


# revision 6
# speedup vs baseline: 1.4434x; 1.0110x over previous
"""MinGRU (B=4, T=4096, D=1024) Trainium2 kernel, 8-core SPMD.

Sharding: core i handles (batch b = i//2, output-channel half j = i%2).
Each core computes u_z = x[b] @ Wz[half].T, u_h = x[b] @ Wh[half].T,
z = sigmoid(u_z + bz), a = 1 - z, bvec = z * (u_h + bh), then the
recurrence h_t = a_t * h_{t-1} + b_t via the hardware tensor_tensor_scan.

Design (all empirically A/B-tested on hardware):
- bf16 matmuls: reads half the SBUF bytes per MAC of fp32r, which holds
  speed under the chip's power-driven clock governor where fp32r drifts
  up (interleaved A/B: bf16 149-152us vs fp32r 159-166us).
- Each 128x128 weight tile streams against two 512-col moving tiles
  into a [128,1024] 2-bank PSUM supertile; _dedupe_ldweights deletes
  the redundant second InstLdweights post-legalization (the in-order PE
  reuses the resident stationary operand - verified bit-correct).
- x fully resident in SBUF (64 KB/partition), loaded once at startup:
  the steady-state loop's only DMA is the bf16 h output.
- One PSUM reader per bank: ACT does z = sigmoid(pz + bz); DVE does
  a = 1 - z, b = (ph + bh) * z, and the scan (fp32 state, bf16 out).
- For_i(staggered_reset=True) removes the per-iteration all-engine
  barrier.
Host pre-transposes x and W so every DMA is row-contiguous and converts
h back to fp32.
"""

import numpy as np

_B, _T, _D = 4, 4096, 1024
_EH = 512
_NG = _EH // 128
_TT = 1024         # timestep supertile (2 PSUM banks)
_NT = _T // _TT    # 4 resident t-supertiles
_NK = _D // 128
_HF = 512


def _dedupe_ldweights(nc):
    from concourse import mybir as mb

    removed = 0
    for fn in nc.m.functions:
        for blk in fn.blocks:
            insts = blk.instructions
            last_sig = None
            dels = []
            for i, inst in enumerate(insts):
                if isinstance(inst, mb.InstLdweights):
                    ap = inst.ins[0]
                    sig = (str(ap.memref), ap.offset, str(ap.ap),
                           str(ap.dtype), inst.perf_mode, inst.tile_position)
                    si = inst.sync_info
                    clean = si is None or (not si.on_wait and not si.on_update)
                    if sig == last_sig and clean:
                        dels.append(i)
                    else:
                        last_sig = sig
                elif isinstance(inst, mb.InstMatmult):
                    if inst.ldweights is not False:
                        last_sig = None
            for i in reversed(dels):
                del insts[i]
            removed += len(dels)
    return removed


def _build(reps=1, loop_n=None):
    from contextlib import ExitStack
    from concourse import bacc, mybir, tile

    f32 = mybir.dt.float32
    bf16 = mybir.dt.float16
    AF = mybir.ActivationFunctionType
    OP = mybir.AluOpType

    nc = bacc.Bacc("TRN2", debug=False, num_devices=8)
    xt = nc.dram_tensor("xt", [_D, _T], bf16, kind="ExternalInput").ap()
    wzt = nc.dram_tensor("wzt", [_D, _EH], bf16, kind="ExternalInput").ap()
    wht = nc.dram_tensor("wht", [_D, _EH], bf16, kind="ExternalInput").ap()
    bzt = nc.dram_tensor("bzt", [128, _NG], f32, kind="ExternalInput").ap()
    bht = nc.dram_tensor("bht", [128, _NG], f32, kind="ExternalInput").ap()
    hout = nc.dram_tensor("h", [_EH, _T], bf16, kind="ExternalOutput").ap()

    with tile.TileContext(nc) as tc, ExitStack() as ctx:
        wpool = ctx.enter_context(tc.tile_pool(name="w", bufs=1))
        vpool = ctx.enter_context(tc.tile_pool(name="v", bufs=3))
        hpool = ctx.enter_context(tc.tile_pool(name="h", bufs=2))
        ppool = ctx.enter_context(tc.tile_pool(name="p", bufs=2, space="PSUM"))

        # x fully resident: 4 supertiles of [128, (k tt)] bf16 = 64 KB/part.
        xres = []
        for t2 in range(_NT):
            xres_t = wpool.tile([128, _NK * _TT], bf16, tag=f"x{t2}")
            xres.append(xres_t)
        wz_sb = wpool.tile([128, _NK * _EH], bf16, tag="wz")
        wh_sb = wpool.tile([128, _NK * _EH], bf16, tag="wh")
        bz_sb = wpool.tile([128, _NG], f32, tag="bz")
        bzn_sb = wpool.tile([128, _NG], f32, tag="bzn")
        bh_sb = wpool.tile([128, _NG], f32, tag="bh")

        def x_chunk(t2, ks, nk):
            nc.sync.dma_start(
                xres[t2][:, ks * _TT:(ks + nk) * _TT].rearrange(
                    "p (k t) -> p k t", k=nk),
                xt.rearrange("(k p) t -> p k t", p=128)[
                    :, ks:ks + nk, t2 * _TT:(t2 + 1) * _TT],
            )

        def w_chunk(k):
            nc.sync.dma_start(
                wz_sb[:, k * _EH:(k + 1) * _EH],
                wzt[k * 128:(k + 1) * 128, :])
            nc.sync.dma_start(
                wh_sb[:, k * _EH:(k + 1) * _EH],
                wht[k * 128:(k + 1) * 128, :])

        x_chunk(0, 0, 4)
        w_chunk(0)
        w_chunk(1)
        nc.sync.dma_start(bz_sb[:], bzt)
        nc.sync.dma_start(bh_sb[:], bht)
        x_chunk(0, 4, 4)
        for k in range(2, _NK):
            w_chunk(k)
        for t2 in range(1, _NT):
            x_chunk(t2, 0, 4)
            x_chunk(t2, 4, 4)
        nc.scalar.mul(bzn_sb[:], bz_sb[:], -1.0)

        def body(first):
          hprev = [None] * _NG
          for t2 in range(_NT):
            xs = xres[t2]
            for g in range(_NG):
                last = (t2 == _NT - 1 and g == _NG - 1)
                pz = ppool.tile([128, _TT], f32, tag="pz")
                ph = ppool.tile([128, _TT], f32, tag="ph")
                for k in range(_NK):
                    for c0 in (0, _HF):
                        nc.tensor.matmul(
                            pz[:, c0:c0 + _HF],
                            lhsT=wz_sb[:, k * _EH + g * 128:
                                       k * _EH + (g + 1) * 128],
                            rhs=xs[:, k * _TT + c0: k * _TT + c0 + _HF],
                            start=(k == 0),
                            stop=(k == _NK - 1),
                        )
                for k in range(_NK):
                    for c0 in (0, _HF):
                        nc.tensor.matmul(
                            ph[:, c0:c0 + _HF],
                            lhsT=wh_sb[:, k * _EH + g * 128:
                                       k * _EH + (g + 1) * 128],
                            rhs=xs[:, k * _TT + c0: k * _TT + c0 + _HF],
                            start=(k == 0),
                            stop=(k == _NK - 1),
                        )
                z = vpool.tile([128, _TT], bf16, tag="z")
                av = vpool.tile([128, _TT], bf16, tag="a")
                bv = vpool.tile([128, _TT], bf16, tag="b")
                hb = hpool.tile([128, _TT], bf16, tag=f"h{g}")
                init = 0.0 if hprev[g] is None \
                    else hprev[g][:, _TT - 1:_TT]
                halves = ((0, _HF), (_HF, _HF)) if last else ((0, _TT),)
                for (c0, w) in halves:
                    sl = slice(c0, c0 + w)
                    nc.scalar.activation(z[:, sl], pz[:, sl], AF.Sigmoid,
                                         bias=bz_sb[:, g:g + 1])
                    nc.scalar.activation(av[:, sl], pz[:, sl], AF.Sigmoid,
                                         bias=bzn_sb[:, g:g + 1],
                                         scale=-1.0)
                    nc.vector.scalar_tensor_tensor(
                        bv[:, sl], ph[:, sl], bh_sb[:, g:g + 1], z[:, sl],
                        OP.add, OP.mult)
                    nc.vector.tensor_tensor_scan(
                        hb[:, sl], av[:, sl], bv[:, sl], init,
                        OP.mult, OP.add)
                    init = hb[:, c0 + w - 1:c0 + w]
                    nc.sync.dma_start(
                        hout[g * 128:(g + 1) * 128,
                             t2 * _TT + c0: t2 * _TT + c0 + w],
                        hb[:, sl])
                hprev[g] = hb

        if loop_n is not None:
            body(True)
            from concourse import mybir as _mb
            with tc.For_i(0, loop_n, 1, hint_engines=(
                    _mb.EngineType.PE, _mb.EngineType.SP,
                    _mb.EngineType.DVE, _mb.EngineType.Activation),
                    staggered_reset=True):
                body(False)
        else:
            for rep in range(reps):
                body(rep == 0)

    _dedupe_ldweights(nc)
    nc.compile()
    return nc


_NC_CACHE = None


def _shard_inputs(inputs):
    bf16 = np.float16
    x = np.asarray(inputs["x"], dtype=np.float32)
    Wz = np.asarray(inputs["Wz"], dtype=np.float32)
    bz = np.asarray(inputs["bz"], dtype=np.float32)
    Wh = np.asarray(inputs["Wh"], dtype=np.float32)
    bh = np.asarray(inputs["bh"], dtype=np.float32)

    wzT = np.ascontiguousarray(Wz.T).astype(bf16)
    whT = np.ascontiguousarray(Wh.T).astype(bf16)

    in_maps = []
    for i in range(8):
        b, j = i // 2, i % 2
        sl = slice(j * _EH, (j + 1) * _EH)
        in_maps.append({
            "xt": np.ascontiguousarray(x[b].T).astype(bf16),
            "wzt": np.ascontiguousarray(wzT[:, sl]),
            "wht": np.ascontiguousarray(whT[:, sl]),
            "bzt": np.ascontiguousarray(bz[sl].reshape(_NG, 128).T),
            "bht": np.ascontiguousarray(bh[sl].reshape(_NG, 128).T),
        })
    return in_maps


def run(inputs, trace=False, tmpdir=None):
    global _NC_CACHE
    from concourse.bass_utils import run_bass_kernel_spmd

    if _NC_CACHE is None:
        _NC_CACHE = _build()
    nc = _NC_CACHE
    in_maps = _shard_inputs(inputs)
    res = run_bass_kernel_spmd(
        nc, in_maps, core_ids=list(range(8)), trace=trace, tmpdir=tmpdir)
    out = np.empty((_B, _T, _D), dtype=np.float32)
    for i in range(8):
        b, j = i // 2, i % 2
        out[b, :, j * _EH:(j + 1) * _EH] = res.results[i]["h"].astype(np.float32).T
    return out, res


def kernel(**inputs):
    out, _ = run(inputs, trace=False)
    return out
